# revision 14
# baseline (speedup 1.0000x reference)
"""DTW distance kernel for Trainium2 (8 NeuronCores, SPMD data-parallel over batch).

Per core: NB=16 batch elements. Host precomputes (cheap input marshalling):
  xm2 = -2x (bf16, [F, nb, T]), yb = y (bf16), y2[j] = sum_f y^2 (bf16 row),
  x2[i] = sum_f x^2 (f32, chunk-column layout), plus a constants blob
  (partition-shift matrix, BIG/zero columns, ones row).
Phase 1 (cost matrix): cost[b][i,j] = ||x[b,:,i] - y[b,:,j]||_2.
  d2 = (-2x)^T y + 1*y2[j] via accumulated bf16 PE matmuls per 128-row chunk;
  x2[i] enters as the ACT sqrt's per-partition bias: cost = sqrt(psum + x2col).
  ACT writes bf16 staging -> DRAM scratch whose per-block layout is skewed by
  the strip lag (skew absorbed into a linear DMA stride), then streamed back
  into a 4-window SBUF ring in big per-window DMAs.
Phase 2 (DP): dtw wavefront. 8 column-strips x 16 batches = 128 partitions
  (partition p = s*16 + b). Strip s lags strip s-1 by L steps. Per step t
  (strip s handles row i = t - L*s), only TWO DVE ops:
    m          = min(Rp[:,1:W+1], Rp[:,0:W])                      (DVE)
    R[:,1:W+1] = scan(state=min(m,state)+cost; state0=R-pad)      (DVE)
  R[:, slot, 0] is a pad column: the PE shift matmul (partition shift by 16)
  moves the previous strip's last-column values into PSUM; one ACT op per
  kb=4 steps copies them (with +BIG bias on strip-0 partitions) into the pad
  slots. m picks up LEFT via col 0; the scan's init reads the DIAG pad.
  Inactive strip lanes stay at ~1e30 ("BIG"); head pads are memset BIG.
"""
import sys
import numpy as np

sys.path.insert(0, "/opt/trn_rl_repo")

import concourse.bass as bass  # noqa: E402
import concourse.bacc as bacc  # noqa: E402
import concourse.mybir as mybir  # noqa: E402
import concourse.tile as tile  # noqa: E402

NCORES = 8
B_FULL, F_FULL, T_FULL = 128, 128, 512
BIG = 1.0e30


def build_dtw(nb, F, T, S, W, L, nslot=64, wt=64, kb=4, nring=4):
    """Per-core SPMD Bass graph. Partition p = s*nb + b."""
    assert S * W == T and S * nb <= 128 and nslot % kb == 0
    P = S * nb
    NC = (T + 127) // 128
    CM = T // NC
    assert CM * NC == T
    TS = L * (S - 1) + T                  # DP steps
    TR = nring * wt                       # costdp ring length
    f32, bf16 = mybir.dt.float32, mybir.dt.bfloat16
    mn, ad = mybir.AluOpType.min, mybir.AluOpType.add
    AF = mybir.ActivationFunctionType

    nc = bacc.Bacc(None, target_bir_lowering=False, debug=False)
    xm2_in = nc.declare_dram_parameter("xm2", [F, nb, T], bf16, isOutput=False)
    yb_in = nc.declare_dram_parameter("yb", [F, nb, T], bf16, isOutput=False)
    y2_in = nc.declare_dram_parameter("y2a", [1, nb * T], bf16, isOutput=False)
    x2_in = nc.declare_dram_parameter("x2a", [CM, nb * NC], f32, isOutput=False)
    # cst cols: 0..127 shift matrix SH (SH[q,p]=1 iff p=q+nb), 128 bigcol
    # (BIG for p<nb else 0), 129 zcol (0 for p<nb else BIG)
    cst = nc.declare_dram_parameter("cst", [128, 130], f32, isOutput=False)
    orow = nc.declare_dram_parameter("orow", [1, T], bf16, isOutput=False)
    out = nc.declare_dram_parameter("out", [nb, 1], f32, isOutput=True)
    scratch = nc.dram_tensor("scratch", [P * TS * W], bf16)

    def scr_ap(offset, dims):
        return bass.AP(tensor=scratch, offset=offset, ap=[list(d) for d in dims])

    with tile.TileContext(nc) as tc:
        with (
            tc.tile_pool(name="persist", bufs=1) as pp,
            tc.tile_pool(name="stg", bufs=4) as stgp,
            tc.tile_pool(name="m", bufs=4) as mp,
            tc.tile_pool(name="ps_d2", bufs=4, space="PSUM") as psd,
            tc.tile_pool(name="ps_b", bufs=3, space="PSUM") as psbp,
        ):
            # ---- constants / persistent state ----
            cstt = pp.tile([128, 130], f32, tag="cstt")
            nc.sync.dma_start(cstt[:], cst[:])
            shmat = cstt[:, 0:128]
            bigcol = cstt[:, 128:129]
            zcol = cstt[:, 129:130]
            ort = pp.tile([1, T], bf16, tag="ort")
            nc.sync.dma_start(ort[:], orow[:])
            y2all = pp.tile([1, nb * T], bf16, tag="y2all")
            nc.sync.dma_start(y2all[:], y2_in[:])
            x2all = pp.tile([CM, nb * NC], f32, tag="x2all")
            nc.sync.dma_start(x2all[:], x2_in[:])

            costdp = pp.tile([P, TR, W], bf16, tag="costdp")
            # head pads: slots [0, L*(S-1)) can be read by inactive-strip
            # steps before any window write covers them. Later wrapped reads
            # of stale slots only feed dead lanes.
            nc.gpsimd.memset(costdp[:, 0:L * (S - 1), :], BIG)
            R = pp.tile([P, nslot, W + 1], f32, tag="R")
            # only slot nslot-1 (pslot of t=0) and the col-0 pads of the
            # first steps (before boundary ACT writes start at slot 2*kb-1)
            # are ever read before being written
            nc.gpsimd.memset(R[:, nslot - 1, 0:W + 1], BIG)
            nc.gpsimd.memset(R[:, 0:2 * kb - 1, 0:1], BIG)

            # ---- inputs: grouped bf16 loads on two DMA rings ----
            GL = 4
            ngrp = (nb + GL - 1) // GL
            xmg = [pp.tile([F, GL, T], bf16, tag=f"xmg{g}", name=f"xmg{g}")
                   for g in range(ngrp)]
            ybg = [pp.tile([F, GL, T], bf16, tag=f"ybg{g}", name=f"ybg{g}")
                  for g in range(ngrp)]
            # group 0 gets its own ring: DMA descriptors of one ring execute
            # concurrently across the engine pool, so every DMA on a shared
            # ring completes near the END of the whole batch. Isolating the
            # first group lets its produce_chunk start ~7us earlier.
            nc.gpsimd.dma_start(xmg[0][:], xm2_in[:, 0:GL, :])
            nc.gpsimd.dma_start(ybg[0][:], yb_in[:, 0:GL, :])
            for g in range(1, ngrp):
                nc.sync.dma_start(xmg[g][:], xm2_in[:, g * GL:(g + 1) * GL, :])
                nc.scalar.dma_start(ybg[g][:], yb_in[:, g * GL:(g + 1) * GL, :])

            def produce_chunk(c, b):
                xm = xmg[b // GL][:, b % GL, :]
                yv = ybg[b // GL][:, b % GL, :]
                ps = psd.tile([CM, T], f32, tag="ps")
                H = T // 2
                for h in range(2):
                    nc.tensor.matmul(
                        ps[:, h * H:(h + 1) * H],
                        xm[:, c * CM:(c + 1) * CM],
                        yv[:, h * H:(h + 1) * H], start=True, stop=False)
                    nc.tensor.matmul(
                        ps[:, h * H:(h + 1) * H],
                        ort[0:1, c * CM:(c + 1) * CM],
                        y2all[0:1, b * T + h * H:b * T + (h + 1) * H],
                        start=False, stop=True)
                stg = stgp.tile([CM, T], bf16, tag="stg")
                nc.scalar.activation(
                    stg[:], ps[:], AF.Sqrt,
                    bias=x2all[:, b * NC + c:b * NC + c + 1], scale=1.0)
                # write skewed: addr(b; i, s, f) =
                #   (s*nb+b)*TS*W + (L*s + c*CM + i)*W + f
                nc.sync.dma_start(
                    scr_ap(b * TS * W + c * CM * W,
                           [[W, CM], [(nb * TS + L) * W, S], [1, W]]),
                    stg[:])

            for b in range(nb):
                produce_chunk(0, b)

            # ---- boundary: shift raw strip-boundary values into R pads ----
            def emit_boundary(i):
                # covers steps u in [kb*i, kb*i+kb); A_u = Rlast[p-nb](u-L)
                # lands in R[p, (u-1)%nslot, 0]; strip-0 rows get +BIG bias.
                s0 = (kb * i - L) % nslot
                psb = psbp.tile([P, kb], f32, tag="psb")
                if s0 + kb <= nslot:
                    nc.tensor.matmul(psb[:], shmat[0:P, 0:P],
                                     R[:, s0:s0 + kb, W:W + 1],
                                     start=True, stop=True)
                else:
                    k1 = nslot - s0
                    nc.tensor.matmul(psb[:, 0:k1], shmat[0:P, 0:P],
                                     R[:, s0:nslot, W:W + 1],
                                     start=True, stop=True)
                    nc.tensor.matmul(psb[:, k1:kb], shmat[0:P, 0:P],
                                     R[:, 0:kb - k1, W:W + 1],
                                     start=True, stop=True)
                sA = (kb * i - 1) % nslot
                if sA + kb <= nslot:
                    nc.scalar.activation(R[:, sA:sA + kb, 0:1], psb[:],
                                         AF.Identity, bias=bigcol, scale=1.0)
                else:
                    k1 = nslot - sA
                    nc.scalar.activation(R[:, sA:nslot, 0:1], psb[:, 0:k1],
                                         AF.Identity, bias=bigcol, scale=1.0)
                    nc.scalar.activation(R[:, 0:kb - k1, 0:1], psb[:, k1:kb],
                                         AF.Identity, bias=bigcol, scale=1.0)

            def dp_step(t):
                slot, pslot = t % nslot, (t - 1) % nslot
                m = mp.tile([P, W], f32, tag="m")
                nc.vector.tensor_tensor(
                    m[:], R[:, pslot, 1:W + 1], R[:, pslot, 0:W], op=mn)
                init = (zcol if t == 0 else R[:, (t - 2) % nslot, 0:1])
                nc.vector.tensor_tensor_scan(
                    R[:, slot, 1:W + 1], m[:], costdp[:, t % TR, :],
                    init, op0=mn, op1=ad)
                # emit the boundary batch whose last source is this step's scan
                u = t + L - (kb - 1)
                if u >= 2 * kb and u % kb == 0 and u < TS:
                    emit_boundary(u // kb)

            # per-window: produce needed chunks, prefetch next window's read,
            # then run this window's DP steps (chunk 0 produced in stage A)
            n_win = (TS + wt - 1) // wt
            prod_c = 1

            def win_read(w):
                t0, t1 = w * wt, min((w + 1) * wt, TS)
                r0 = t0 % TR
                # strip s valid rows cover t in [L*s, L*s+T)
                full = [s for s in range(S)
                        if L * s <= t0 and L * s + T >= t1]
                if full:
                    s_a, s_b = min(full), max(full)
                    half = (s_b - s_a + 1) // 2
                    if w == 0 and half > 0:
                        # first window gates the DP start: split across rings
                        nc.sync.dma_start(
                            costdp[s_a * nb:(s_a + half) * nb,
                                   r0:r0 + (t1 - t0), :],
                            scr_ap(s_a * nb * TS * W + t0 * W,
                                   [[TS * W, half * nb],
                                    [1, (t1 - t0) * W]]))
                        nc.scalar.dma_start(
                            costdp[(s_a + half) * nb:(s_b + 1) * nb,
                                   r0:r0 + (t1 - t0), :],
                            scr_ap((s_a + half) * nb * TS * W + t0 * W,
                                   [[TS * W, (s_b - s_a + 1 - half) * nb],
                                    [1, (t1 - t0) * W]]))
                    else:
                        nc.sync.dma_start(
                            costdp[s_a * nb:(s_b + 1) * nb,
                                   r0:r0 + (t1 - t0), :],
                            scr_ap(s_a * nb * TS * W + t0 * W,
                                   [[TS * W, (s_b - s_a + 1) * nb],
                                    [1, (t1 - t0) * W]]))
                for s in range(S):
                    if s in full:
                        continue
                    v0, v1 = max(t0, L * s), min(t1, L * s + T)
                    if v0 >= v1:
                        continue
                    nc.sync.dma_start(
                        costdp[s * nb:(s + 1) * nb,
                               r0 + (v0 - t0):r0 + (v1 - t0), :],
                        scr_ap(s * nb * TS * W + v0 * W,
                               [[TS * W, nb], [1, (v1 - v0) * W]]))

            PF = nring - 2

            def need_c(v):
                t1p = min((v + 1) * wt, TS)
                return min(NC - 1, (t1p - 1) // CM)

            read_done = 0
            prods = []            # queued (c, b) productions
            prods_done = nb       # chunk 0 produced during stage A
            for w in range(n_win):
                # queue chunks needed one window beyond the read target
                tgt = need_c(min(w + PF + 1, n_win - 1))
                while prod_c <= tgt:
                    prods.extend((prod_c, b) for b in range(nb))
                    prod_c += 1
                # read any window (up to w+PF) whose chunks are produced
                while (read_done <= min(w + PF, n_win - 1)
                       and prods_done >= (need_c(read_done) + 1) * nb):
                    win_read(read_done)
                    read_done += 1
                for t in range(w * wt, min((w + 1) * wt, TS)):
                    dp_step(t)
                    if t % 8 == 2 and prods:
                        produce_chunk(*prods.pop(0))
                        prods_done += 1

            # ---- extract answers: strip S-1, row T-1, col W ----
            nc.sync.dma_start(
                out[:], R[(S - 1) * nb:P, (TS - 1) % nslot, W:W + 1])

    nc.compile()
    return nc


_cache = {}

NB = B_FULL // NCORES
S_CFG, W_CFG, L_CFG = 8, 64, 6


def _get_nc():
    key = "full"
    if key not in _cache:
        _cache[key] = build_dtw(
            nb=NB, F=F_FULL, T=T_FULL, S=S_CFG, W=W_CFG, L=L_CFG)
    return _cache[key]


def _make_consts():
    nb = NB
    cstv = np.zeros((128, 130), np.float32)
    for q in range(128 - nb):
        cstv[q, q + nb] = 1.0            # SH[q, p]: p = q + nb
    cstv[:nb, 128] = BIG                 # bigcol
    cstv[nb:, 129] = BIG                 # zcol (0 for p<nb)
    import ml_dtypes
    return cstv, np.ones((1, T_FULL), ml_dtypes.bfloat16)


def make_in_maps(x, y):
    """Shard FULL (B,F,T) inputs into per-core in_maps with host-side
    preprocessing: transpose to [F, nb, T], cast to bf16, precompute the
    squared-norm rows/columns the cost matrix needs."""
    import ml_dtypes
    bf16 = ml_dtypes.bfloat16
    nb, T, NCc = NB, T_FULL, 4
    CM = T // NCc
    cstv, orv = _make_consts()
    in_maps = []
    for c in range(NCORES):
        xs = np.ascontiguousarray(
            x[c * nb:(c + 1) * nb].transpose(1, 0, 2), dtype=np.float32)
        ys = np.ascontiguousarray(
            y[c * nb:(c + 1) * nb].transpose(1, 0, 2), dtype=np.float32)
        xm2 = np.ascontiguousarray((-2.0 * xs).astype(bf16))
        yb = np.ascontiguousarray(ys.astype(bf16))
        # match device numerics: squares of the bf16-cast values, f32 sums
        xm2f = xm2.astype(np.float32)
        ybf = yb.astype(np.float32)
        x2 = (xm2f * xm2f).sum(axis=0) * 0.25          # [nb, T]
        y2 = (ybf * ybf).sum(axis=0)                   # [nb, T]
        y2a = y2.reshape(1, nb * T).astype(bf16)
        # x2a[i, b*NC+c] = x2[b, c*CM+i]
        x2a = np.ascontiguousarray(
            x2.reshape(nb, NCc, CM).transpose(2, 0, 1).reshape(CM, nb * NCc)
        ).astype(np.float32)
        in_maps.append({"xm2": xm2, "yb": yb, "y2a": y2a, "x2a": x2a,
                        "cst": cstv, "orow": orv})
    return in_maps


def kernel(x, y):
    from concourse.bass_utils import run_bass_kernel_spmd

    x = np.ascontiguousarray(x, dtype=np.float32)
    y = np.ascontiguousarray(y, dtype=np.float32)
    nc = _get_nc()
    res = run_bass_kernel_spmd(nc, make_in_maps(x, y), list(range(NCORES)))
    outs = [res.results[c]["out"].reshape(NB) for c in range(NCORES)]
    return np.concatenate(outs).astype(np.float32)


# revision 16
# speedup vs baseline: 1.0055x; 1.0055x over previous
"""DTW distance kernel for Trainium2 (8 NeuronCores, SPMD data-parallel over batch).

Per core: NB=16 batch elements. Host precomputes (cheap input marshalling):
  xm2 = -2x (bf16, [F, nb, T]), yb = y (bf16), y2[j] = sum_f y^2 (bf16 row),
  x2[i] = sum_f x^2 (f32, chunk-column layout), plus a constants blob
  (partition-shift matrix, BIG/zero columns, ones row).
Phase 1 (cost matrix): cost[b][i,j] = ||x[b,:,i] - y[b,:,j]||_2.
  d2 = (-2x)^T y + 1*y2[j] via accumulated bf16 PE matmuls per 128-row chunk;
  x2[i] enters as the ACT sqrt's per-partition bias: cost = sqrt(psum + x2col).
  ACT writes bf16 staging -> DRAM scratch whose per-block layout is skewed by
  the strip lag (skew absorbed into a linear DMA stride), then streamed back
  into a 4-window SBUF ring in big per-window DMAs.
Phase 2 (DP): dtw wavefront. 8 column-strips x 16 batches = 128 partitions
  (partition p = s*16 + b). Strip s lags strip s-1 by L steps. Per step t
  (strip s handles row i = t - L*s), only TWO DVE ops:
    m          = min(Rp[:,1:W+1], Rp[:,0:W])                      (DVE)
    R[:,1:W+1] = scan(state=min(m,state)+cost; state0=R-pad)      (DVE)
  R[:, slot, 0] is a pad column: the PE shift matmul (partition shift by 16)
  moves the previous strip's last-column values into PSUM; one ACT op per
  kb=4 steps copies them (with +BIG bias on strip-0 partitions) into the pad
  slots. m picks up LEFT via col 0; the scan's init reads the DIAG pad.
  Inactive strip lanes stay at ~1e30 ("BIG"); head pads are memset BIG.
"""
import sys
import numpy as np

sys.path.insert(0, "/opt/trn_rl_repo")

import concourse.bass as bass  # noqa: E402
import concourse.bacc as bacc  # noqa: E402
import concourse.mybir as mybir  # noqa: E402
import concourse.tile as tile  # noqa: E402

NCORES = 8
B_FULL, F_FULL, T_FULL = 128, 128, 512
BIG = 1.0e30


def build_dtw(nb, F, T, S, W, L, nslot=64, wt=64, kb=4, nring=4):
    """Per-core SPMD Bass graph. Partition p = s*nb + b."""
    assert S * W == T and S * nb <= 128 and nslot % kb == 0
    P = S * nb
    NC = (T + 127) // 128
    CM = T // NC
    assert CM * NC == T
    TS = L * (S - 1) + T                  # DP steps
    TR = nring * wt                       # costdp ring length
    f32, bf16 = mybir.dt.float32, mybir.dt.bfloat16
    mn, ad = mybir.AluOpType.min, mybir.AluOpType.add
    AF = mybir.ActivationFunctionType

    nc = bacc.Bacc(None, target_bir_lowering=False, debug=False)
    xm2_in = nc.declare_dram_parameter("xm2", [F, nb, T], bf16, isOutput=False)
    yb_in = nc.declare_dram_parameter("yb", [F, nb, T], bf16, isOutput=False)
    y2_in = nc.declare_dram_parameter("y2a", [1, nb * T], bf16, isOutput=False)
    x2_in = nc.declare_dram_parameter("x2a", [CM, nb * NC], f32, isOutput=False)
    # cst cols: 0..127 shift matrix SH (SH[q,p]=1 iff p=q+nb), 128 bigcol
    # (BIG for p<nb else 0), 129 zcol (0 for p<nb else BIG)
    cst = nc.declare_dram_parameter("cst", [128, 130], f32, isOutput=False)
    orow = nc.declare_dram_parameter("orow", [1, T], bf16, isOutput=False)
    out = nc.declare_dram_parameter("out", [nb, 1], f32, isOutput=True)
    scratch = nc.dram_tensor("scratch", [P * TS * W], bf16)

    def scr_ap(offset, dims):
        return bass.AP(tensor=scratch, offset=offset, ap=[list(d) for d in dims])

    with tile.TileContext(nc) as tc:
        with (
            tc.tile_pool(name="persist", bufs=1) as pp,
            tc.tile_pool(name="stg", bufs=4) as stgp,
            tc.tile_pool(name="m", bufs=4) as mp,
            tc.tile_pool(name="ps_d2", bufs=4, space="PSUM") as psd,
            tc.tile_pool(name="ps_b", bufs=3, space="PSUM") as psbp,
        ):
            # ---- constants / persistent state ----
            cstt = pp.tile([128, 130], f32, tag="cstt")
            nc.sync.dma_start(cstt[:], cst[:])
            shmat = cstt[:, 0:128]
            bigcol = cstt[:, 128:129]
            zcol = cstt[:, 129:130]
            ort = pp.tile([1, T], bf16, tag="ort")
            nc.sync.dma_start(ort[:], orow[:])
            y2all = pp.tile([1, nb * T], bf16, tag="y2all")
            nc.sync.dma_start(y2all[:], y2_in[:])
            x2all = pp.tile([CM, nb * NC], f32, tag="x2all")
            nc.sync.dma_start(x2all[:], x2_in[:])

            costdp = pp.tile([P, TR, W], bf16, tag="costdp")
            # head pads: slots [0, L*(S-1)) can be read by inactive-strip
            # steps before any window write covers them. Later wrapped reads
            # of stale slots only feed dead lanes.
            nc.gpsimd.memset(costdp[:, 0:L * (S - 1), :], BIG)
            R = pp.tile([P, nslot, W + 1], f32, tag="R")
            # only slot nslot-1 (pslot of t=0) and the col-0 pads of the
            # first steps (before boundary ACT writes start at slot 2*kb-1)
            # are ever read before being written
            nc.gpsimd.memset(R[:, nslot - 1, 0:W + 1], BIG)
            nc.gpsimd.memset(R[:, 0:2 * kb - 1, 0:1], BIG)

            # ---- inputs: grouped bf16 loads on two DMA rings ----
            GL = 4
            ngrp = (nb + GL - 1) // GL
            xmg = [pp.tile([F, GL, T], bf16, tag=f"xmg{g}", name=f"xmg{g}")
                   for g in range(ngrp)]
            ybg = [pp.tile([F, GL, T], bf16, tag=f"ybg{g}", name=f"ybg{g}")
                  for g in range(ngrp)]
            for g in range(ngrp):
                nc.sync.dma_start(xmg[g][:], xm2_in[:, g * GL:(g + 1) * GL, :])
                nc.scalar.dma_start(ybg[g][:], yb_in[:, g * GL:(g + 1) * GL, :])

            def produce_chunk(c, b):
                xm = xmg[b // GL][:, b % GL, :]
                yv = ybg[b // GL][:, b % GL, :]
                ps = psd.tile([CM, T], f32, tag="ps")
                H = T // 2
                for h in range(2):
                    nc.tensor.matmul(
                        ps[:, h * H:(h + 1) * H],
                        xm[:, c * CM:(c + 1) * CM],
                        yv[:, h * H:(h + 1) * H], start=True, stop=False)
                    nc.tensor.matmul(
                        ps[:, h * H:(h + 1) * H],
                        ort[0:1, c * CM:(c + 1) * CM],
                        y2all[0:1, b * T + h * H:b * T + (h + 1) * H],
                        start=False, stop=True)
                stg = stgp.tile([CM, T], bf16, tag="stg")
                nc.scalar.activation(
                    stg[:], ps[:], AF.Sqrt,
                    bias=x2all[:, b * NC + c:b * NC + c + 1], scale=1.0)
                # write skewed: addr(b; i, s, f) =
                #   (s*nb+b)*TS*W + (L*s + c*CM + i)*W + f
                nc.sync.dma_start(
                    scr_ap(b * TS * W + c * CM * W,
                           [[W, CM], [(nb * TS + L) * W, S], [1, W]]),
                    stg[:])

            for b in range(nb):
                produce_chunk(0, b)

            # ---- boundary: shift raw strip-boundary values into R pads ----
            def emit_boundary(i):
                # covers steps u in [kb*i, kb*i+kb); A_u = Rlast[p-nb](u-L)
                # lands in R[p, (u-1)%nslot, 0]; strip-0 rows get +BIG bias.
                s0 = (kb * i - L) % nslot
                psb = psbp.tile([P, kb], f32, tag="psb")
                if s0 + kb <= nslot:
                    nc.tensor.matmul(psb[:], shmat[0:P, 0:P],
                                     R[:, s0:s0 + kb, W:W + 1],
                                     start=True, stop=True)
                else:
                    k1 = nslot - s0
                    nc.tensor.matmul(psb[:, 0:k1], shmat[0:P, 0:P],
                                     R[:, s0:nslot, W:W + 1],
                                     start=True, stop=True)
                    nc.tensor.matmul(psb[:, k1:kb], shmat[0:P, 0:P],
                                     R[:, 0:kb - k1, W:W + 1],
                                     start=True, stop=True)
                sA = (kb * i - 1) % nslot
                if sA + kb <= nslot:
                    nc.scalar.activation(R[:, sA:sA + kb, 0:1], psb[:],
                                         AF.Identity, bias=bigcol, scale=1.0)
                else:
                    k1 = nslot - sA
                    nc.scalar.activation(R[:, sA:nslot, 0:1], psb[:, 0:k1],
                                         AF.Identity, bias=bigcol, scale=1.0)
                    nc.scalar.activation(R[:, 0:kb - k1, 0:1], psb[:, k1:kb],
                                         AF.Identity, bias=bigcol, scale=1.0)

            def dp_step(t):
                slot, pslot = t % nslot, (t - 1) % nslot
                m = mp.tile([P, W], f32, tag="m")
                nc.vector.tensor_tensor(
                    m[:], R[:, pslot, 1:W + 1], R[:, pslot, 0:W], op=mn)
                init = (zcol if t == 0 else R[:, (t - 2) % nslot, 0:1])
                nc.vector.tensor_tensor_scan(
                    R[:, slot, 1:W + 1], m[:], costdp[:, t % TR, :],
                    init, op0=mn, op1=ad)
                # emit the boundary batch whose last source is this step's scan
                u = t + L - (kb - 1)
                if u >= 2 * kb and u % kb == 0 and u < TS:
                    emit_boundary(u // kb)

            # per-window: produce needed chunks, prefetch next window's read,
            # then run this window's DP steps (chunk 0 produced in stage A)
            n_win = (TS + wt - 1) // wt
            prod_c = 1

            def win_read(w):
                t0, t1 = w * wt, min((w + 1) * wt, TS)
                r0 = t0 % TR
                # strip s valid rows cover t in [L*s, L*s+T)
                full = [s for s in range(S)
                        if L * s <= t0 and L * s + T >= t1]
                if full:
                    s_a, s_b = min(full), max(full)
                    half = (s_b - s_a + 1) // 2
                    if w == 0 and half > 0:
                        # first window gates the DP start: split across rings
                        nc.sync.dma_start(
                            costdp[s_a * nb:(s_a + half) * nb,
                                   r0:r0 + (t1 - t0), :],
                            scr_ap(s_a * nb * TS * W + t0 * W,
                                   [[TS * W, half * nb],
                                    [1, (t1 - t0) * W]]))
                        nc.scalar.dma_start(
                            costdp[(s_a + half) * nb:(s_b + 1) * nb,
                                   r0:r0 + (t1 - t0), :],
                            scr_ap((s_a + half) * nb * TS * W + t0 * W,
                                   [[TS * W, (s_b - s_a + 1 - half) * nb],
                                    [1, (t1 - t0) * W]]))
                    else:
                        nc.sync.dma_start(
                            costdp[s_a * nb:(s_b + 1) * nb,
                                   r0:r0 + (t1 - t0), :],
                            scr_ap(s_a * nb * TS * W + t0 * W,
                                   [[TS * W, (s_b - s_a + 1) * nb],
                                    [1, (t1 - t0) * W]]))
                for s in range(S):
                    if s in full:
                        continue
                    v0, v1 = max(t0, L * s), min(t1, L * s + T)
                    if v0 >= v1:
                        continue
                    nc.sync.dma_start(
                        costdp[s * nb:(s + 1) * nb,
                               r0 + (v0 - t0):r0 + (v1 - t0), :],
                        scr_ap(s * nb * TS * W + v0 * W,
                               [[TS * W, nb], [1, (v1 - v0) * W]]))

            PF = nring - 2

            def need_c(v):
                t1p = min((v + 1) * wt, TS)
                return min(NC - 1, (t1p - 1) // CM)

            read_done = 0
            prods = []            # queued (c, b) productions
            prods_done = nb       # chunk 0 produced during stage A
            for w in range(n_win):
                # queue chunks needed one window beyond the read target
                tgt = need_c(min(w + PF + 1, n_win - 1))
                while prod_c <= tgt:
                    prods.extend((prod_c, b) for b in range(nb))
                    prod_c += 1
                # read any window (up to w+PF) whose chunks are produced
                while (read_done <= min(w + PF, n_win - 1)
                       and prods_done >= (need_c(read_done) + 1) * nb):
                    win_read(read_done)
                    read_done += 1
                for t in range(w * wt, min((w + 1) * wt, TS)):
                    dp_step(t)
                    if t % 4 == 2 and prods:
                        produce_chunk(*prods.pop(0))
                        prods_done += 1

            # ---- extract answers: strip S-1, row T-1, col W ----
            nc.sync.dma_start(
                out[:], R[(S - 1) * nb:P, (TS - 1) % nslot, W:W + 1])

    nc.compile()
    return nc


_cache = {}

NB = B_FULL // NCORES
S_CFG, W_CFG, L_CFG = 8, 64, 6


def _get_nc():
    key = "full"
    if key not in _cache:
        _cache[key] = build_dtw(
            nb=NB, F=F_FULL, T=T_FULL, S=S_CFG, W=W_CFG, L=L_CFG)
    return _cache[key]


def _make_consts():
    nb = NB
    cstv = np.zeros((128, 130), np.float32)
    for q in range(128 - nb):
        cstv[q, q + nb] = 1.0            # SH[q, p]: p = q + nb
    cstv[:nb, 128] = BIG                 # bigcol
    cstv[nb:, 129] = BIG                 # zcol (0 for p<nb)
    import ml_dtypes
    return cstv, np.ones((1, T_FULL), ml_dtypes.bfloat16)


def make_in_maps(x, y):
    """Shard FULL (B,F,T) inputs into per-core in_maps with host-side
    preprocessing: transpose to [F, nb, T], cast to bf16, precompute the
    squared-norm rows/columns the cost matrix needs."""
    import ml_dtypes
    bf16 = ml_dtypes.bfloat16
    nb, T, NCc = NB, T_FULL, 4
    CM = T // NCc
    cstv, orv = _make_consts()
    in_maps = []
    for c in range(NCORES):
        xs = np.ascontiguousarray(
            x[c * nb:(c + 1) * nb].transpose(1, 0, 2), dtype=np.float32)
        ys = np.ascontiguousarray(
            y[c * nb:(c + 1) * nb].transpose(1, 0, 2), dtype=np.float32)
        xm2 = np.ascontiguousarray((-2.0 * xs).astype(bf16))
        yb = np.ascontiguousarray(ys.astype(bf16))
        # match device numerics: squares of the bf16-cast values, f32 sums
        xm2f = xm2.astype(np.float32)
        ybf = yb.astype(np.float32)
        x2 = (xm2f * xm2f).sum(axis=0) * 0.25          # [nb, T]
        y2 = (ybf * ybf).sum(axis=0)                   # [nb, T]
        y2a = y2.reshape(1, nb * T).astype(bf16)
        # x2a[i, b*NC+c] = x2[b, c*CM+i]
        x2a = np.ascontiguousarray(
            x2.reshape(nb, NCc, CM).transpose(2, 0, 1).reshape(CM, nb * NCc)
        ).astype(np.float32)
        in_maps.append({"xm2": xm2, "yb": yb, "y2a": y2a, "x2a": x2a,
                        "cst": cstv, "orow": orv})
    return in_maps


def kernel(x, y):
    from concourse.bass_utils import run_bass_kernel_spmd

    x = np.ascontiguousarray(x, dtype=np.float32)
    y = np.ascontiguousarray(y, dtype=np.float32)
    nc = _get_nc()
    res = run_bass_kernel_spmd(nc, make_in_maps(x, y), list(range(NCORES)))
    outs = [res.results[c]["out"].reshape(NB) for c in range(NCORES)]
    return np.concatenate(outs).astype(np.float32)
